# revision 18
# baseline (speedup 1.0000x reference)
"""Multi-head attention (B=8, S=2048, D=512, H=8) on 8 Trainium2 NeuronCores.

Strategy: pure data parallelism - one batch element per core, no collectives.

Per-core pipeline (matmuls fp16, fp32 PSUM):
  ScalarE is the hard floor: 256 exp tiles of [128,1024] ~= 272 us busy, and
  exp only runs on the Activation engine.  Everything else is scheduled to
  hide under it, with the Tensor engine kept continuously busy so its DVFS
  p-state stays at full clock:
    - minimal head phase: k-proj (et0), q-proj (et0, first s-half), v-proj;
      attention starts as soon as those are done.
    - all remaining projection matmuls (k et1-3, q rest, final proj) are
      interleaved 1 unit/slot into the attention jt loop as PE filler.
    - ScalarE does exp ONLY (bias adds moved to DVE tensor_scalar ops, so the
      Exp activation table is loaded exactly once).
    - PSUM->SBUF drains of the PV result go to GpSimd (Pool) to keep DVE
      under budget (DVE does the 256 mask-muls = its main load).
  maskT is stored per-s-half ([128,16,1024] fp16) and reloaded in place at
  the sh boundary to fit SBUF.  Output is written fp16 (rel-err budget 2e-2).

Softmax: reference softmax(where(mask==0,-1e30,s)) == exp(s)*mask normalized;
scores are O(1) so no max-subtraction; 0/1 mask exact in fp16.  Scale
1/sqrt(dk)=1/8 folded into Wq/bq on host.
"""
import numpy as np

import concourse.bacc as bacc
import concourse.bass as bass
import concourse.mybir as mybir
import concourse.tile as tile
from concourse.bass_utils import run_bass_kernel_spmd

B, S, D, H, DK = 8, 2048, 512, 8, 64
P = 128            # partition tile
NET = D // P       # 4 e-tiles (contraction chunks / head pairs)
NST = S // P       # 16 s-tiles / j-tiles
SCW = 512          # matmul moving free dim (1 PSUM bank)
NSC = S // SCW     # 4
SHW = 1024         # attention s-block width (2 PSUM banks)
NSH = S // SHW     # 2

f32 = mybir.dt.float32
fp16 = mybir.dt.float16
fp8 = mybir.dt.float8e4
DR = mybir.MatmulPerfMode.DoubleRow

_CACHE: dict = {}


def _build():
    nc = bacc.Bacc("TRN2", target_bir_lowering=False, debug=False)

    d_xq = nc.dram_tensor("xq", [D, S], fp16, kind="ExternalInput")
    d_xk = nc.dram_tensor("xk", [D, S], fp16, kind="ExternalInput")
    d_xv = nc.dram_tensor("xv", [D, S], fp16, kind="ExternalInput")
    d_mskT = nc.dram_tensor("mskT", [S, S], fp16, kind="ExternalInput")
    d_wq = nc.dram_tensor("wq", [D, D], fp16, kind="ExternalInput")  # Wq.T
    d_wk = nc.dram_tensor("wk", [D, D], fp16, kind="ExternalInput")  # Wk.T
    d_wv = nc.dram_tensor("wv", [D, D], fp16, kind="ExternalInput")  # Wv.T
    d_wo = nc.dram_tensor("wo", [D, D], fp16, kind="ExternalInput")  # Wo.T
    d_bq = nc.dram_tensor("bq", [D], f32, kind="ExternalInput")      # bq/8
    d_bk = nc.dram_tensor("bk", [D], f32, kind="ExternalInput")
    d_bv = nc.dram_tensor("bv", [D], f32, kind="ExternalInput")
    d_bo = nc.dram_tensor("bo", [D], f32, kind="ExternalInput")
    d_out = nc.dram_tensor("out", [S, D], fp16, kind="ExternalOutput")
    d_rec = nc.dram_tensor("rec_dram", [H, S], fp16)

    Exp = mybir.ActivationFunctionType.Exp
    Add = mybir.AluOpType.add

    with tile.TileContext(nc) as tc, \
         tc.tile_pool(name="persist", bufs=1) as persist, \
         tc.tile_pool(name="xpool", bufs=1) as xpool, \
         tc.tile_pool(name="wpool", bufs=1) as wpool, \
         tc.tile_pool(name="projps", bufs=2, space="PSUM") as projps, \
         tc.tile_pool(name="attn", bufs=4) as attn, \
         tc.tile_pool(name="attnps", bufs=2, space="PSUM") as attnps:

        # q/k in fp8e4 for DoubleRow scores matmuls; dim2 is the k-subtile
        # pair (second subtile zeroed - contraction is only dk=64 deep)
        qT = persist.tile([P, NET, 2, S], fp8)           # [e%128, et, kt, m]
        kT = persist.tile([P, NET, 2, S], fp8)
        v_aug = persist.tile([P, NST, H, DK + 1], fp16)  # [j%128, jt, h, d|1]
        outT = persist.tile([P, NET, S], fp16)           # [hd%128, et, m]
        maskT = persist.tile([P, NST, SHW], fp16)        # current s-half only
        denom = persist.tile([P, NSH, 64], f32)
        bq_sb = persist.tile([P, NET], f32)
        bk_sb = persist.tile([P, NET], f32)
        bv_bc = persist.tile([P, D], f32)
        wo_sb = persist.tile([P, NET, D], fp16)
        bo_bc = persist.tile([P, D], f32)

        # x / w inputs (k, q, v alive simultaneously early on)
        xk_sb = xpool.tile([P, NET, S], fp16, name="xk_sb")
        xq_sb = xpool.tile([P, NET, S], fp16, name="xq_sb")
        xv_sb = xpool.tile([P, NET, S], fp16, name="xv_sb")
        wk_sb = wpool.tile([P, NET, D], fp16, name="wk_sb")
        wq_sb = wpool.tile([P, NET, D], fp16, name="wq_sb")
        wv_sb = wpool.tile([P, NET, D], fp16, name="wv_sb")

        msk_ap = d_mskT.ap().rearrange("(jt p) s -> p jt s", p=P)
        xk_ap = d_xk.ap().rearrange("(cc p) s -> p cc s", p=P)
        xq_ap = d_xq.ap().rearrange("(cc p) s -> p cc s", p=P)
        xv_ap = d_xv.ap().rearrange("(cc p) s -> p cc s", p=P)

        # ---- input DMAs, priority order ----
        # mask for the first couple of jt, then xk / xq first-half / xv
        # (weights are small; biases tiny)
        nc.sync.dma_start(out=bq_sb, in_=d_bq.ap().rearrange("(cc p) -> p cc", p=P))
        nc.sync.dma_start(out=bk_sb, in_=d_bk.ap().rearrange("(cc p) -> p cc", p=P))
        nc.sync.dma_start(
            out=bv_bc,
            in_=bass.AP(tensor=d_bv.ap().tensor, offset=0, ap=[[0, P], [1, D]]))
        nc.vector.memset(v_aug[:, :, :, DK:DK + 1], 1.0)
        # zero the unused second k-subtile planes of qT/kT (Pool is idle)
        nc.gpsimd.memset(qT[:, :, 1, :], 0.0)
        nc.gpsimd.memset(kT[:, :, 1, :], 0.0)
        for jt in range(2):
            nc.sync.dma_start(out=maskT[:, jt, :], in_=msk_ap[:, jt, 0:SHW])
        nc.sync.dma_start(
            out=wk_sb, in_=d_wk.ap().rearrange("(cc p) e -> p cc e", p=P))
        for sc in range(NSC):
            nc.sync.dma_start(out=xk_sb[:, :, sc * SCW:(sc + 1) * SCW],
                              in_=xk_ap[:, :, sc * SCW:(sc + 1) * SCW])
        nc.sync.dma_start(
            out=wq_sb, in_=d_wq.ap().rearrange("(cc p) e -> p cc e", p=P))
        for sc in range(2):
            nc.sync.dma_start(out=xq_sb[:, :, sc * SCW:(sc + 1) * SCW],
                              in_=xq_ap[:, :, sc * SCW:(sc + 1) * SCW])
        nc.sync.dma_start(
            out=wv_sb, in_=d_wv.ap().rearrange("(cc p) e -> p cc e", p=P))
        for jt in range(2, 6):
            nc.sync.dma_start(out=maskT[:, jt, :], in_=msk_ap[:, jt, 0:SHW])
        for sc in range(NSC):
            nc.sync.dma_start(out=xv_sb[:, :, sc * SCW:(sc + 1) * SCW],
                              in_=xv_ap[:, :, sc * SCW:(sc + 1) * SCW])
        for jt in range(6, NST):
            nc.sync.dma_start(out=maskT[:, jt, :], in_=msk_ap[:, jt, 0:SHW])

        # ---- projection unit emitters ----
        def qk_pair(dst, x_sb, w_sb, bias, et, sc0):
            # transposed layout [e, m]: stationary = weight cols, moving = x.
            # Two s-chunks share each stationary load (ldweights amortize).
            psA = projps.tile([P, SCW], f32, tag="pp", name="ppA")
            psB = projps.tile([P, SCW], f32, tag="pp", name="ppB")
            for cc in range(NET):
                w_sl = w_sb[:, cc, et * P:(et + 1) * P]
                nc.tensor.matmul(
                    psA, w_sl, x_sb[:, cc, sc0 * SCW:(sc0 + 1) * SCW],
                    start=(cc == 0), stop=(cc == NET - 1))
                nc.tensor.matmul(
                    psB, w_sl, x_sb[:, cc, (sc0 + 1) * SCW:(sc0 + 2) * SCW],
                    start=(cc == 0), stop=(cc == NET - 1))
            for ps, sc in ((psA, sc0), (psB, sc0 + 1)):
                nc.vector.tensor_scalar(
                    dst[:, et, 0, sc * SCW:(sc + 1) * SCW], ps,
                    bias[:, et:et + 1], None, Add)

        def v_unit(st):
            # natural layout [j, e] into v_aug rows for j-tile st
            ps = projps.tile([P, SCW], f32, tag="pp", name="pp")
            for cc in range(NET):
                nc.tensor.matmul(
                    ps,
                    xv_sb[:, cc, st * P:(st + 1) * P],
                    wv_sb[:, cc, :],
                    start=(cc == 0), stop=(cc == NET - 1))
            nc.vector.tensor_add(
                v_aug[:, st, :, 0:DK],
                ps.rearrange("p (h d) -> p h d", h=H),
                bv_bc.rearrange("p (h d) -> p h d", h=H))

        def final_unit(st):
            # out rows [st*128, (st+1)*128) @ Wo.T + bo -> DMA (fp16, 2 rings)
            ps = projps.tile([P, D], f32, tag="pp", name="pp")
            for cc in range(NET):
                nc.tensor.matmul(
                    ps,
                    outT[:, cc, st * P:(st + 1) * P],
                    wo_sb[:, cc, :],
                    start=(cc == 0), stop=(cc == NET - 1))
            o_sb = attn.tile([P, D], fp16, tag="os", bufs=2, name="o_sb")
            nc.vector.tensor_add(o_sb, ps, bo_bc)
            half = D // 2
            nc.sync.dma_start(
                out=d_out.ap()[st * P:(st + 1) * P, 0:half],
                in_=o_sb[:, 0:half])
            nc.sync.dma_start(
                out=d_out.ap()[st * P:(st + 1) * P, half:D],
                in_=o_sb[:, half:D])

        # ---- head phase: k et0, q et0 (first half), v all ----
        for sc0 in (0, 2):
            qk_pair(kT, xk_sb, wk_sb, bk_sb, 0, sc0)
        qk_pair(qT, xq_sb, wq_sb, bq_sb, 0, 0)
        for st in range(NST):
            v_unit(st)
        # wo / bo loads (needed from sh-boundary onwards)
        nc.sync.dma_start(
            out=wo_sb, in_=d_wo.ap().rearrange("(cc p) e -> p cc e", p=P))
        nc.sync.dma_start(
            out=bo_bc,
            in_=bass.AP(tensor=d_bo.ap().tensor, offset=0, ap=[[0, P], [1, D]]))
        # remaining xq loads (second half, needed by sh=1)
        for sc in range(2, NSC):
            nc.sync.dma_start(out=xq_sb[:, :, sc * SCW:(sc + 1) * SCW],
                              in_=xq_ap[:, :, sc * SCW:(sc + 1) * SCW])

        # ---- filler queue: remaining proj work, interleaved into slots ----
        # each entry is one stationary-shared pair (~2.1 us of PE work), so
        # space them out every few slots to keep per-slot PE load even.
        filler = []
        for et in range(1, NET):
            for sc0 in (0, 2):
                filler.append(lambda et=et, sc0=sc0:
                              qk_pair(kT, xk_sb, wk_sb, bk_sb, et, sc0))
            filler.append(lambda et=et:
                          qk_pair(qT, xq_sb, wq_sb, bq_sb, et, 0))
        for et in range(NET):
            filler.append(lambda et=et:
                          qk_pair(qT, xq_sb, wq_sb, bq_sb, et, 2))
        filler = [(fn, 3) for fn in filler]  # (emitter, slot spacing)
        fill_i = [0]
        cool = [0]

        def emit_filler():
            if cool[0] > 0:
                cool[0] -= 1
                return
            if fill_i[0] < len(filler):
                fn, spacing = filler[fill_i[0]]
                fill_i[0] += 1
                cool[0] = spacing - 1
                if fn is not None:
                    fn()

        # ---- attention ----
        for sh in range(NSH):
            c0 = sh * SHW
            for h in range(H):
                et, ro = h // 2, 64 * (h % 2)
                pv0 = attnps.tile([65, SCW], f32, tag="pv", bufs=2, name="pv0")
                pv1 = attnps.tile([65, SCW], f32, tag="pv", bufs=2, name="pv1")
                pvs = (pv0, pv1)
                for jt in range(NST):
                    sc_ps = attnps.tile([P, SHW], f32, tag="sc", bufs=2,
                                        name="sc_ps")
                    for i in range(2):
                        nc.tensor.matmul(
                            sc_ps[:, i * SCW:(i + 1) * SCW],
                            kT[ro:ro + DK, et, :, jt * P:(jt + 1) * P],
                            qT[ro:ro + DK, et, :,
                               c0 + i * SCW:c0 + (i + 1) * SCW],
                            start=True, stop=True, perf_mode=DR)
                    ex = attn.tile([P, SHW], fp16, tag="ex", bufs=4, name="ex")
                    # 1/sqrt(dk)=1/8 applied here (q/k kept unscaled so the
                    # fp8 quantization stays in the normal-number range)
                    nc.scalar.activation(ex, sc_ps, Exp, scale=0.125)
                    pb = attn.tile([P, SHW], fp16, tag="pb", bufs=4, name="pb")
                    nc.vector.tensor_mul(pb, ex, maskT[:, jt, :])
                    for i in range(2):
                        nc.tensor.matmul(
                            pvs[i], v_aug[:, jt, h, :],
                            pb[:, i * SCW:(i + 1) * SCW],
                            start=(jt == 0), stop=(jt == NST - 1))
                    emit_filler()
                    if sh == 0 and h == 7:
                        # reload mask second half in place (WAR-tracked):
                        # mul(sh0,h7,jt) was the last reader of this slice.
                        nc.sync.dma_start(out=maskT[:, jt, :],
                                          in_=msk_ap[:, jt, SHW:S])
                # drain pv -> outT rows + denominator row (DVE; neither
                # GPSIMD nor DMA can read PSUM)
                for i in range(2):
                    cols = c0 + i * SCW
                    nc.vector.tensor_copy(
                        outT[ro:ro + DK, et, cols:cols + SCW], pvs[i][0:DK, :])
                    dst_t = attn.tile([1, SCW], f32, tag="dst", bufs=2,
                                      name="dst_t")
                    nc.vector.tensor_copy(dst_t, pvs[i][64:65, :])
                    pbase = h * 16 + i * 8
                    nc.gpsimd.dma_start(
                        out=denom[pbase:pbase + 8, sh, :], in_=dst_t)
                if h % 2 == 1:
                    # pair (2et, 2et+1) done for this half: normalize in place
                    rec = attn.tile([32, 64], fp16, tag="rec", bufs=2,
                                    name="rec")
                    with nc.allow_low_precision(reason="softmax recip fp16 ok"):
                        nc.vector.reciprocal(
                            rec, denom[et * 32:(et + 1) * 32, sh, :])
                    nc.sync.dma_start(
                        out=d_rec.ap()[2 * et:2 * et + 2, c0:c0 + SHW],
                        in_=rec)
                    rb = attn.tile([P, SHW], fp16, tag="rb", bufs=2,
                                   name="rb")
                    nc.gpsimd.dma_start(
                        out=rb[0:64, :],
                        in_=bass.AP(tensor=d_rec.ap().tensor,
                                    offset=(2 * et) * S + c0,
                                    ap=[[0, 64], [1, SHW]]))
                    nc.gpsimd.dma_start(
                        out=rb[64:128, :],
                        in_=bass.AP(tensor=d_rec.ap().tensor,
                                    offset=(2 * et + 1) * S + c0,
                                    ap=[[0, 64], [1, SHW]]))
                    nc.vector.tensor_mul(outT[:, et, c0:c0 + SHW],
                                         outT[:, et, c0:c0 + SHW], rb)
            # final projection for this s-half (sh0: as filler during sh1,
            # delayed a few slots so norm(sh0, et3) lands first)
            if sh == 0:
                filler.append((None, 6))
                for st in range(0, NST // NSH):
                    filler.append((lambda st=st: final_unit(st), 2))
        for st in range(NST // NSH, NST):
            final_unit(st)
        # drain any unemitted filler (shouldn't happen, but be safe)
        while fill_i[0] < len(filler):
            fn, _ = filler[fill_i[0]]
            fill_i[0] += 1
            if fn is not None:
                fn()

    nc.compile()
    return nc


def _get_nc():
    if "nc" not in _CACHE:
        _CACHE["nc"] = _build()
    return _CACHE["nc"]


def _preprocess(Q, K, V, mask, Wq, bq, Wk, bk, Wv, bv, Wo, bo):
    """Host-side sharding + layout marshalling (per-core input dicts)."""
    mT = np.ascontiguousarray(np.asarray(mask)[0, 0].T).astype(np.float16)
    wq_h = np.ascontiguousarray(np.asarray(Wq).T).astype(np.float16)
    wk_h = np.ascontiguousarray(np.asarray(Wk).T).astype(np.float16)
    wv_h = np.ascontiguousarray(np.asarray(Wv).T).astype(np.float16)
    wo_h = np.ascontiguousarray(np.asarray(Wo).T).astype(np.float16)
    bq_h = np.asarray(bq, dtype=np.float32)
    bk_h = np.asarray(bk, dtype=np.float32)
    bv_h = np.asarray(bv, dtype=np.float32)
    bo_h = np.asarray(bo, dtype=np.float32)
    Q, K, V = np.asarray(Q), np.asarray(K), np.asarray(V)
    in_maps = []
    for b in range(B):
        in_maps.append({
            "xq": np.ascontiguousarray(Q[b].T).astype(np.float16),
            "xk": np.ascontiguousarray(K[b].T).astype(np.float16),
            "xv": np.ascontiguousarray(V[b].T).astype(np.float16),
            "mskT": mT,
            "wq": wq_h, "wk": wk_h, "wv": wv_h, "wo": wo_h,
            "bq": bq_h, "bk": bk_h, "bv": bv_h, "bo": bo_h,
        })
    return in_maps


def run(inputs: dict, trace: bool = False):
    nc = _get_nc()
    in_maps = _preprocess(**inputs)
    res = run_bass_kernel_spmd(nc, in_maps, core_ids=list(range(B)), trace=trace)
    outp = np.stack([res.results[b]["out"] for b in range(B)], axis=0)
    return outp.astype(np.float32), res


def kernel(**inputs) -> np.ndarray:
    outp, _ = run(inputs, trace=False)
    return outp


# revision 24
# speedup vs baseline: 1.3415x; 1.3415x over previous
"""Multi-head attention (B=8, S=2048, D=512, H=8) on 8 Trainium2 NeuronCores.

Strategy: pure data parallelism - one batch element per core, no collectives.

Per-core pipeline (matmuls fp16, fp32 PSUM):
  ScalarE is the hard floor: 256 exp tiles of [128,1024] ~= 272 us busy, and
  exp only runs on the Activation engine.  Everything else is scheduled to
  hide under it, with the Tensor engine kept continuously busy so its DVFS
  p-state stays at full clock:
    - minimal head phase: k-proj (et0), q-proj (et0, first s-half), v-proj;
      attention starts as soon as those are done.
    - all remaining projection matmuls (k et1-3, q rest, final proj) are
      interleaved 1 unit/slot into the attention jt loop as PE filler.
    - ScalarE does exp ONLY (bias adds moved to DVE tensor_scalar ops, so the
      Exp activation table is loaded exactly once).
    - PSUM->SBUF drains of the PV result go to GpSimd (Pool) to keep DVE
      under budget (DVE does the 256 mask-muls = its main load).
  maskT is stored per-s-half ([128,16,1024] fp16) and reloaded in place at
  the sh boundary to fit SBUF.  Output is written fp16 (rel-err budget 2e-2).

Softmax: reference softmax(where(mask==0,-1e30,s)) == exp(s)*mask normalized;
scores are O(1) so no max-subtraction; 0/1 mask exact in fp16.  Scale
1/sqrt(dk)=1/8 folded into Wq/bq on host.
"""
import numpy as np

import concourse.bacc as bacc
import concourse.bass as bass
import concourse.mybir as mybir
import concourse.tile as tile
from concourse.bass_utils import run_bass_kernel_spmd

B, S, D, H, DK = 8, 2048, 512, 8, 64
P = 128            # partition tile
NET = D // P       # 4 e-tiles (contraction chunks / head pairs)
NST = S // P       # 16 s-tiles / j-tiles
SCW = 512          # matmul moving free dim (1 PSUM bank)
NSC = S // SCW     # 4
SHW = 1024         # attention s-block width (2 PSUM banks)
NSH = S // SHW     # 2

f32 = mybir.dt.float32
fp16 = mybir.dt.float16
fp8 = mybir.dt.float8e4
DR = mybir.MatmulPerfMode.DoubleRow

_CACHE: dict = {}


def _build():
    nc = bacc.Bacc("TRN2", target_bir_lowering=False, debug=False)

    d_xq = nc.dram_tensor("xq", [D, S], fp16, kind="ExternalInput")
    d_xk = nc.dram_tensor("xk", [D, S], fp16, kind="ExternalInput")
    d_xv = nc.dram_tensor("xv", [D, S], fp16, kind="ExternalInput")
    d_mskT = nc.dram_tensor("mskT", [S, S], fp16, kind="ExternalInput")
    d_wq = nc.dram_tensor("wq", [D, D], fp16, kind="ExternalInput")  # Wq.T
    d_wk = nc.dram_tensor("wk", [D, D], fp16, kind="ExternalInput")  # Wk.T
    d_wv = nc.dram_tensor("wv", [D, D], fp16, kind="ExternalInput")  # Wv.T
    d_wo = nc.dram_tensor("wo", [D, D], fp16, kind="ExternalInput")  # Wo.T
    d_bq = nc.dram_tensor("bq", [D], f32, kind="ExternalInput")      # bq/8
    d_bk = nc.dram_tensor("bk", [D], f32, kind="ExternalInput")
    d_bv = nc.dram_tensor("bv", [D], f32, kind="ExternalInput")
    d_bo = nc.dram_tensor("bo", [D], f32, kind="ExternalInput")
    d_out = nc.dram_tensor("out", [S, D], fp16, kind="ExternalOutput")
    d_rec = nc.dram_tensor("rec_dram", [H, S], fp16)

    Exp = mybir.ActivationFunctionType.Exp
    Add = mybir.AluOpType.add

    with tile.TileContext(nc) as tc, \
         tc.tile_pool(name="persist", bufs=1) as persist, \
         tc.tile_pool(name="xpool", bufs=1) as xpool, \
         tc.tile_pool(name="wpool", bufs=1) as wpool, \
         tc.tile_pool(name="projps", bufs=2, space="PSUM") as projps, \
         tc.tile_pool(name="attn", bufs=4) as attn, \
         tc.tile_pool(name="attnps", bufs=2, space="PSUM") as attnps:

        # q/k in fp8e4 for DoubleRow scores matmuls (contraction padded to
        # 256 with zeros so the PE fast path engages):
        #   qT: paired layout [e%128, et, m] (moving side - the k-subtile
        #       dim is added as a stride-0 AP dim; rows of the other head
        #       are multiplied by stationary zeros)
        #   kT: flat per-head planes [128, h*S + m]; head h's dk values
        #       live in rows (h%2)*64..+64 (so the DVE conversion from the
        #       paired projection PSUM stays partition-aligned), all other
        #       rows zero; one extra S-wide zero plane at the end serves as
        #       the k-subtile-1 operand for every head.
        qT = persist.tile([P, NET, S], fp8)              # [e%128, et, m]
        kT = persist.tile([P, (H + 1) * S], fp8)
        v_aug = persist.tile([P, NST, H, DK + 1], fp16)  # [j%128, jt, h, d|1]
        outT = persist.tile([P, NET, S], fp16)           # [hd%128, et, m]
        maskT = persist.tile([P, NST, SHW], fp16)        # current s-half only
        denom = persist.tile([P, NSH, 64], f32)
        bq_sb = persist.tile([P, NET], f32)
        bk_sb = persist.tile([P, NET], f32)
        bv_bc = persist.tile([P, D], f32)
        wo_sb = persist.tile([P, NET, D], fp16)
        bo_bc = persist.tile([P, D], f32)

        # x / w inputs (k, q, v alive simultaneously early on)
        xk_sb = xpool.tile([P, NET, S], fp16, name="xk_sb")
        xq_sb = xpool.tile([P, NET, S], fp16, name="xq_sb")
        xv_sb = xpool.tile([P, NET, S], fp16, name="xv_sb")
        wk_sb = wpool.tile([P, NET, D], fp16, name="wk_sb")
        wq_sb = wpool.tile([P, NET, D], fp16, name="wq_sb")
        wv_sb = wpool.tile([P, NET, D], fp16, name="wv_sb")

        msk_ap = d_mskT.ap().rearrange("(jt p) s -> p jt s", p=P)
        xk_ap = d_xk.ap().rearrange("(cc p) s -> p cc s", p=P)
        xq_ap = d_xq.ap().rearrange("(cc p) s -> p cc s", p=P)
        xv_ap = d_xv.ap().rearrange("(cc p) s -> p cc s", p=P)

        # ---- input DMAs, priority order ----
        # mask for the first couple of jt, then xk / xq first-half / xv
        # (weights are small; biases tiny)
        nc.sync.dma_start(out=bq_sb, in_=d_bq.ap().rearrange("(cc p) -> p cc", p=P))
        nc.sync.dma_start(out=bk_sb, in_=d_bk.ap().rearrange("(cc p) -> p cc", p=P))
        nc.sync.dma_start(
            out=bv_bc,
            in_=bass.AP(tensor=d_bv.ap().tensor, offset=0, ap=[[0, P], [1, D]]))
        nc.vector.memset(v_aug[:, :, :, DK:DK + 1], 1.0)
        # zero all of kT once (Pool is idle early); conversions then fill
        # only each head's data rows
        nc.gpsimd.memset(kT, 0.0)
        for jt in range(2):
            nc.sync.dma_start(out=maskT[:, jt, :], in_=msk_ap[:, jt, 0:SHW])
        nc.sync.dma_start(
            out=wk_sb, in_=d_wk.ap().rearrange("(cc p) e -> p cc e", p=P))
        for sc in range(NSC):
            nc.sync.dma_start(out=xk_sb[:, :, sc * SCW:(sc + 1) * SCW],
                              in_=xk_ap[:, :, sc * SCW:(sc + 1) * SCW])
        nc.sync.dma_start(
            out=wq_sb, in_=d_wq.ap().rearrange("(cc p) e -> p cc e", p=P))
        for sc in range(2):
            nc.sync.dma_start(out=xq_sb[:, :, sc * SCW:(sc + 1) * SCW],
                              in_=xq_ap[:, :, sc * SCW:(sc + 1) * SCW])
        nc.sync.dma_start(
            out=wv_sb, in_=d_wv.ap().rearrange("(cc p) e -> p cc e", p=P))
        for jt in range(2, 6):
            nc.sync.dma_start(out=maskT[:, jt, :], in_=msk_ap[:, jt, 0:SHW])
        for sc in range(NSC):
            nc.sync.dma_start(out=xv_sb[:, :, sc * SCW:(sc + 1) * SCW],
                              in_=xv_ap[:, :, sc * SCW:(sc + 1) * SCW])
        for jt in range(6, NST):
            nc.sync.dma_start(out=maskT[:, jt, :], in_=msk_ap[:, jt, 0:SHW])

        # ---- projection unit emitters ----
        def proj_pair_steps(which, et, sc0):
            # transposed layout [e, m]: stationary = weight cols, moving = x.
            # Two s-chunks share each stationary load (ldweights amortize);
            # emitted as 4 per-cc steps (~0.5us each) to keep PE load even.
            if which == "q":
                x_sb, w_sb, bias = xq_sb, wq_sb, bq_sb
            else:
                x_sb, w_sb, bias = xk_sb, wk_sb, bk_sb
            state = {}

            def step(cc):
                def go():
                    if cc == 0:
                        state["A"] = projps.tile([P, SCW], f32, tag="pp",
                                                 name="ppA")
                        state["B"] = projps.tile([P, SCW], f32, tag="pp",
                                                 name="ppB")
                    psA, psB = state["A"], state["B"]
                    w_sl = w_sb[:, cc, et * P:(et + 1) * P]
                    nc.tensor.matmul(
                        psA, w_sl, x_sb[:, cc, sc0 * SCW:(sc0 + 1) * SCW],
                        start=(cc == 0), stop=(cc == NET - 1))
                    nc.tensor.matmul(
                        psB, w_sl, x_sb[:, cc, (sc0 + 1) * SCW:(sc0 + 2) * SCW],
                        start=(cc == 0), stop=(cc == NET - 1))
                    if cc == NET - 1:
                        for ps, sc in ((psA, sc0), (psB, sc0 + 1)):
                            lo, hi = sc * SCW, (sc + 1) * SCW
                            if which == "q":
                                nc.vector.tensor_scalar(
                                    qT[:, et, lo:hi], ps,
                                    bias[:, et:et + 1], None, Add)
                            else:
                                he, ho = 2 * et, 2 * et + 1
                                nc.vector.tensor_scalar(
                                    kT[0:64, he * S + lo:he * S + hi],
                                    ps[0:64, :], bias[0:64, et:et + 1],
                                    None, Add)
                                nc.vector.tensor_scalar(
                                    kT[64:P, ho * S + lo:ho * S + hi],
                                    ps[64:P, :], bias[64:P, et:et + 1],
                                    None, Add)
                return go
            return [step(c) for c in range(NET)]

        def emit_now(steps):
            for s in steps:
                s()

        def v_unit(st):
            # natural layout [j, e] into v_aug rows for j-tile st
            ps = projps.tile([P, SCW], f32, tag="pp", name="pp")
            for cc in range(NET):
                nc.tensor.matmul(
                    ps,
                    xv_sb[:, cc, st * P:(st + 1) * P],
                    wv_sb[:, cc, :],
                    start=(cc == 0), stop=(cc == NET - 1))
            nc.vector.tensor_add(
                v_aug[:, st, :, 0:DK],
                ps.rearrange("p (h d) -> p h d", h=H),
                bv_bc.rearrange("p (h d) -> p h d", h=H))

        def final_unit(st):
            # out rows [st*128, (st+1)*128) @ Wo.T + bo -> DMA (fp16, 2 rings)
            ps = projps.tile([P, D], f32, tag="pp", name="pp")
            for cc in range(NET):
                nc.tensor.matmul(
                    ps,
                    outT[:, cc, st * P:(st + 1) * P],
                    wo_sb[:, cc, :],
                    start=(cc == 0), stop=(cc == NET - 1))
            o_sb = attn.tile([P, D], fp16, tag="os", bufs=2, name="o_sb")
            nc.vector.tensor_add(o_sb, ps, bo_bc)
            half = D // 2
            nc.sync.dma_start(
                out=d_out.ap()[st * P:(st + 1) * P, 0:half],
                in_=o_sb[:, 0:half])
            nc.sync.dma_start(
                out=d_out.ap()[st * P:(st + 1) * P, half:D],
                in_=o_sb[:, half:D])

        # ---- head phase: k et0, q et0 (first half), v all ----
        for sc0 in (0, 2):
            emit_now(proj_pair_steps("k", 0, sc0))
        emit_now(proj_pair_steps("q", 0, 0))
        for st in range(NST):
            v_unit(st)
        # wo / bo loads (needed from sh-boundary onwards)
        nc.sync.dma_start(
            out=wo_sb, in_=d_wo.ap().rearrange("(cc p) e -> p cc e", p=P))
        nc.sync.dma_start(
            out=bo_bc,
            in_=bass.AP(tensor=d_bo.ap().tensor, offset=0, ap=[[0, P], [1, D]]))
        # remaining xq loads (second half, needed by sh=1)
        for sc in range(2, NSC):
            nc.sync.dma_start(out=xq_sb[:, :, sc * SCW:(sc + 1) * SCW],
                              in_=xq_ap[:, :, sc * SCW:(sc + 1) * SCW])

        # ---- filler queue: remaining proj work, interleaved into slots ----
        # entries are ~0.5us per-cc steps, one per slot
        filler = []
        for et in range(1, NET):
            for sc0 in (0, 2):
                filler += proj_pair_steps("k", et, sc0)
            filler += proj_pair_steps("q", et, 0)
        for et in range(NET):
            filler += proj_pair_steps("q", et, 2)
        filler = [(fn, 1) for fn in filler]  # (emitter, slot spacing)
        fill_i = [0]
        cool = [0]

        def emit_filler():
            if cool[0] > 0:
                cool[0] -= 1
                return
            if fill_i[0] < len(filler):
                fn, spacing = filler[fill_i[0]]
                fill_i[0] += 1
                cool[0] = spacing - 1
                if fn is not None:
                    fn()

        # ---- attention ----
        for sh in range(NSH):
            c0 = sh * SHW
            for h in range(H):
                et, ro = h // 2, 64 * (h % 2)
                pv0 = attnps.tile([65, SCW], f32, tag="pv", bufs=2, name="pv0")
                pv1 = attnps.tile([65, SCW], f32, tag="pv", bufs=2, name="pv1")
                pvs = (pv0, pv1)
                for jt in range(NST):
                    sc_ps = attnps.tile([P, SHW], f32, tag="sc", bufs=2,
                                        name="sc_ps")
                    # stationary: head plane [128, 2, 128] whose second
                    # k-subtile is the shared zero plane; moving: paired qT
                    # with a stride-0 k-subtile dim (zero-weight rows kill
                    # the other head's values)
                    k_sl = kT[:, h * S + jt * P:h * S + (jt + 1) * P]
                    lhsT = bass.AP(
                        tensor=k_sl.tensor, offset=k_sl.offset,
                        ap=[k_sl.ap[0], [(H - h) * S, 2], k_sl.ap[-1]])
                    for i in range(2):
                        q_sl = qT[:, et, c0 + i * SCW:c0 + (i + 1) * SCW]
                        rhs = bass.AP(
                            tensor=q_sl.tensor, offset=q_sl.offset,
                            ap=[q_sl.ap[0], [0, 2], q_sl.ap[-1]])
                        nc.tensor.matmul(
                            sc_ps[:, i * SCW:(i + 1) * SCW],
                            lhsT, rhs,
                            start=True, stop=True, perf_mode=DR)
                    ex = attn.tile([P, SHW], fp16, tag="ex", bufs=4, name="ex")
                    # 1/sqrt(dk)=1/8 applied here (q/k kept unscaled so the
                    # fp8 quantization stays in the normal-number range)
                    nc.scalar.activation(ex, sc_ps, Exp, scale=0.125)
                    pb = attn.tile([P, SHW], fp16, tag="pb", bufs=4, name="pb")
                    nc.vector.tensor_mul(pb, ex, maskT[:, jt, :])
                    for i in range(2):
                        nc.tensor.matmul(
                            pvs[i], v_aug[:, jt, h, :],
                            pb[:, i * SCW:(i + 1) * SCW],
                            start=(jt == 0), stop=(jt == NST - 1))
                    emit_filler()
                    if sh == 0 and h == 7:
                        # reload mask second half in place (WAR-tracked):
                        # mul(sh0,h7,jt) was the last reader of this slice.
                        nc.sync.dma_start(out=maskT[:, jt, :],
                                          in_=msk_ap[:, jt, SHW:S])
                # drain pv -> outT rows + denominator row (DVE; neither
                # GPSIMD nor DMA can read PSUM)
                for i in range(2):
                    cols = c0 + i * SCW
                    nc.vector.tensor_copy(
                        outT[ro:ro + DK, et, cols:cols + SCW], pvs[i][0:DK, :])
                    dst_t = attn.tile([1, SCW], f32, tag="dst", bufs=2,
                                      name="dst_t")
                    nc.vector.tensor_copy(dst_t, pvs[i][64:65, :])
                    pbase = h * 16 + i * 8
                    nc.gpsimd.dma_start(
                        out=denom[pbase:pbase + 8, sh, :], in_=dst_t)
                if h % 2 == 1:
                    # pair (2et, 2et+1) done for this half: normalize in place
                    rec = attn.tile([32, 64], fp16, tag="rec", bufs=2,
                                    name="rec")
                    with nc.allow_low_precision(reason="softmax recip fp16 ok"):
                        nc.vector.reciprocal(
                            rec, denom[et * 32:(et + 1) * 32, sh, :])
                    nc.sync.dma_start(
                        out=d_rec.ap()[2 * et:2 * et + 2, c0:c0 + SHW],
                        in_=rec)
                    rb = attn.tile([P, SHW], fp16, tag="rb", bufs=2,
                                   name="rb")
                    nc.gpsimd.dma_start(
                        out=rb[0:64, :],
                        in_=bass.AP(tensor=d_rec.ap().tensor,
                                    offset=(2 * et) * S + c0,
                                    ap=[[0, 64], [1, SHW]]))
                    nc.gpsimd.dma_start(
                        out=rb[64:128, :],
                        in_=bass.AP(tensor=d_rec.ap().tensor,
                                    offset=(2 * et + 1) * S + c0,
                                    ap=[[0, 64], [1, SHW]]))
                    nc.vector.tensor_mul(outT[:, et, c0:c0 + SHW],
                                         outT[:, et, c0:c0 + SHW], rb)
            # final projection for this s-half (sh0: as filler during sh1,
            # delayed a few slots so norm(sh0, et3) lands first)
            if sh == 0:
                filler.append((None, 6))
                for st in range(0, NST // NSH):
                    filler.append((lambda st=st: final_unit(st), 2))
        for st in range(NST // NSH, NST):
            final_unit(st)
        # drain any unemitted filler (shouldn't happen, but be safe)
        while fill_i[0] < len(filler):
            fn, _ = filler[fill_i[0]]
            fill_i[0] += 1
            if fn is not None:
                fn()

    nc.compile()
    return nc


def _get_nc():
    if "nc" not in _CACHE:
        _CACHE["nc"] = _build()
    return _CACHE["nc"]


def _preprocess(Q, K, V, mask, Wq, bq, Wk, bk, Wv, bv, Wo, bo):
    """Host-side sharding + layout marshalling (per-core input dicts)."""
    mT = np.ascontiguousarray(np.asarray(mask)[0, 0].T).astype(np.float16)
    wq_h = np.ascontiguousarray(np.asarray(Wq).T).astype(np.float16)
    wk_h = np.ascontiguousarray(np.asarray(Wk).T).astype(np.float16)
    wv_h = np.ascontiguousarray(np.asarray(Wv).T).astype(np.float16)
    wo_h = np.ascontiguousarray(np.asarray(Wo).T).astype(np.float16)
    bq_h = np.asarray(bq, dtype=np.float32)
    bk_h = np.asarray(bk, dtype=np.float32)
    bv_h = np.asarray(bv, dtype=np.float32)
    bo_h = np.asarray(bo, dtype=np.float32)
    Q, K, V = np.asarray(Q), np.asarray(K), np.asarray(V)
    in_maps = []
    for b in range(B):
        in_maps.append({
            "xq": np.ascontiguousarray(Q[b].T).astype(np.float16),
            "xk": np.ascontiguousarray(K[b].T).astype(np.float16),
            "xv": np.ascontiguousarray(V[b].T).astype(np.float16),
            "mskT": mT,
            "wq": wq_h, "wk": wk_h, "wv": wv_h, "wo": wo_h,
            "bq": bq_h, "bk": bk_h, "bv": bv_h, "bo": bo_h,
        })
    return in_maps


def run(inputs: dict, trace: bool = False):
    nc = _get_nc()
    in_maps = _preprocess(**inputs)
    res = run_bass_kernel_spmd(nc, in_maps, core_ids=list(range(B)), trace=trace)
    outp = np.stack([res.results[b]["out"] for b in range(B)], axis=0)
    return outp.astype(np.float32), res


def kernel(**inputs) -> np.ndarray:
    outp, _ = run(inputs, trace=False)
    return outp


# revision 32
# speedup vs baseline: 1.3912x; 1.0371x over previous
"""Multi-head attention (B=8, S=2048, D=512, H=8) on 8 Trainium2 NeuronCores.

Strategy: pure data parallelism - one batch element per core, no collectives.

Per-core pipeline (matmuls fp16, fp32 PSUM):
  ScalarE is the hard floor: 256 exp tiles of [128,1024] ~= 272 us busy, and
  exp only runs on the Activation engine.  Everything else is scheduled to
  hide under it, with the Tensor engine kept continuously busy so its DVFS
  p-state stays at full clock:
    - minimal head phase: k-proj (et0), q-proj (et0, first s-half), v-proj;
      attention starts as soon as those are done.
    - all remaining projection matmuls (k et1-3, q rest, final proj) are
      interleaved 1 unit/slot into the attention jt loop as PE filler.
    - ScalarE does exp ONLY (bias adds moved to DVE tensor_scalar ops, so the
      Exp activation table is loaded exactly once).
    - PSUM->SBUF drains of the PV result go to GpSimd (Pool) to keep DVE
      under budget (DVE does the 256 mask-muls = its main load).
  maskT is stored per-s-half ([128,16,1024] fp16) and reloaded in place at
  the sh boundary to fit SBUF.  Output is written fp16 (rel-err budget 2e-2).

Softmax: reference softmax(where(mask==0,-1e30,s)) == exp(s)*mask normalized;
scores are O(1) so no max-subtraction; 0/1 mask exact in fp16.  Scale
1/sqrt(dk)=1/8 folded into Wq/bq on host.
"""
import numpy as np

import concourse.bacc as bacc
import concourse.bass as bass
import concourse.mybir as mybir
import concourse.tile as tile
from concourse.bass_utils import run_bass_kernel_spmd

B, S, D, H, DK = 8, 2048, 512, 8, 64
P = 128            # partition tile
NET = D // P       # 4 e-tiles (contraction chunks / head pairs)
NST = S // P       # 16 s-tiles / j-tiles
SCW = 512          # matmul moving free dim (1 PSUM bank)
NSC = S // SCW     # 4
SHW = 1024         # attention s-block width (2 PSUM banks)
NSH = S // SHW     # 2

f32 = mybir.dt.float32
fp16 = mybir.dt.float16
fp8 = mybir.dt.float8e4
DR = mybir.MatmulPerfMode.DoubleRow

_CACHE: dict = {}


def _build():
    nc = bacc.Bacc("TRN2", target_bir_lowering=False, debug=False)

    d_xq = nc.dram_tensor("xq", [D, S], fp16, kind="ExternalInput")
    d_xk = nc.dram_tensor("xk", [D, S], fp16, kind="ExternalInput")
    d_xv = nc.dram_tensor("xv", [D, S], fp16, kind="ExternalInput")
    d_mskT = nc.dram_tensor("mskT", [S, S], fp16, kind="ExternalInput")
    d_wq = nc.dram_tensor("wq", [D, D], fp16, kind="ExternalInput")  # Wq.T
    d_wk = nc.dram_tensor("wk", [D, D], fp16, kind="ExternalInput")  # Wk.T
    d_wv = nc.dram_tensor("wv", [D, D], fp16, kind="ExternalInput")  # Wv.T
    d_wo = nc.dram_tensor("wo", [D, D], fp16, kind="ExternalInput")  # Wo.T
    d_bq = nc.dram_tensor("bq", [D], f32, kind="ExternalInput")      # bq/8
    d_bk = nc.dram_tensor("bk", [D], f32, kind="ExternalInput")
    d_bv = nc.dram_tensor("bv", [D], f32, kind="ExternalInput")
    d_bo = nc.dram_tensor("bo", [D], f32, kind="ExternalInput")
    d_out = nc.dram_tensor("out", [S, D], fp16, kind="ExternalOutput")
    d_rec = nc.dram_tensor("rec_dram", [H, S], fp16)

    Exp = mybir.ActivationFunctionType.Exp
    Add = mybir.AluOpType.add

    with tile.TileContext(nc) as tc, \
         tc.tile_pool(name="persist", bufs=1) as persist, \
         tc.tile_pool(name="xpool", bufs=1) as xpool, \
         tc.tile_pool(name="wpool", bufs=1) as wpool, \
         tc.tile_pool(name="projps", bufs=2, space="PSUM") as projps, \
         tc.tile_pool(name="attn", bufs=4) as attn, \
         tc.tile_pool(name="attnps", bufs=2, space="PSUM") as attnps:

        # q/k in fp8e4 for DoubleRow scores matmuls (contraction padded to
        # 256 with zeros so the PE fast path engages):
        #   qT: paired layout [e%128, et, m] (moving side - the k-subtile
        #       dim is added as a stride-0 AP dim; rows of the other head
        #       are multiplied by stationary zeros)
        #   kT: flat per-head planes [128, h*S + m]; head h's dk values
        #       live in rows (h%2)*64..+64 (so the DVE conversion from the
        #       paired projection PSUM stays partition-aligned), all other
        #       rows zero; one extra S-wide zero plane at the end serves as
        #       the k-subtile-1 operand for every head.
        qT = persist.tile([P, NET, S], fp8)              # [e%128, et, m]
        kT = persist.tile([P, (H + 1) * S], fp8)
        v_aug = persist.tile([P, NST, H, DK + 1], fp16)  # [j%128, jt, h, d|1]
        outT = persist.tile([P, NET, S], fp16)           # [hd%128, et, m]
        maskT = persist.tile([P, NST, SHW], fp16)        # current s-half only
        denom = persist.tile([P, NSH, 64], f32)
        bq_sb = persist.tile([P, NET], f32)
        bk_sb = persist.tile([P, NET], f32)
        bv_bc = persist.tile([P, D], f32)
        wo_sb = persist.tile([P, NET, D], fp16)
        bo_bc = persist.tile([P, D], f32)

        # x / w inputs (k, q, v alive simultaneously early on)
        xk_sb = xpool.tile([P, NET, S], fp16, name="xk_sb")
        xq_sb = xpool.tile([P, NET, S], fp16, name="xq_sb")
        xv_sb = xpool.tile([P, NET, S], fp16, name="xv_sb")
        wk_sb = wpool.tile([P, NET, D], fp16, name="wk_sb")
        wq_sb = wpool.tile([P, NET, D], fp16, name="wq_sb")
        wv_sb = wpool.tile([P, NET, D], fp16, name="wv_sb")

        msk_ap = d_mskT.ap().rearrange("(jt p) s -> p jt s", p=P)
        xk_ap = d_xk.ap().rearrange("(cc p) s -> p cc s", p=P)
        xq_ap = d_xq.ap().rearrange("(cc p) s -> p cc s", p=P)
        xv_ap = d_xv.ap().rearrange("(cc p) s -> p cc s", p=P)

        # ---- input DMAs, priority order ----
        # mask for the first couple of jt, then xk / xq first-half / xv
        # (weights are small; biases tiny)
        nc.sync.dma_start(out=bq_sb, in_=d_bq.ap().rearrange("(cc p) -> p cc", p=P))
        nc.sync.dma_start(out=bk_sb, in_=d_bk.ap().rearrange("(cc p) -> p cc", p=P))
        nc.sync.dma_start(
            out=bv_bc,
            in_=bass.AP(tensor=d_bv.ap().tensor, offset=0, ap=[[0, P], [1, D]]))
        nc.vector.memset(v_aug[:, :, :, DK:DK + 1], 1.0)
        # zero all of kT once (Pool is idle early); conversions then fill
        # only each head's data rows
        nc.gpsimd.memset(kT, 0.0)
        # input DMA priority: x tensors first (projections gate attention
        # start), mask slices trail (mask jt is first read at slot ~jt)
        nc.sync.dma_start(
            out=wk_sb, in_=d_wk.ap().rearrange("(cc p) e -> p cc e", p=P))
        nc.sync.dma_start(
            out=wq_sb, in_=d_wq.ap().rearrange("(cc p) e -> p cc e", p=P))
        for sc in range(2):
            nc.sync.dma_start(out=xk_sb[:, :, sc * SCW:(sc + 1) * SCW],
                              in_=xk_ap[:, :, sc * SCW:(sc + 1) * SCW])
        for sc in range(2):
            nc.sync.dma_start(out=xq_sb[:, :, sc * SCW:(sc + 1) * SCW],
                              in_=xq_ap[:, :, sc * SCW:(sc + 1) * SCW])
        for sc in range(2, NSC):
            nc.sync.dma_start(out=xk_sb[:, :, sc * SCW:(sc + 1) * SCW],
                              in_=xk_ap[:, :, sc * SCW:(sc + 1) * SCW])
        nc.sync.dma_start(
            out=wv_sb, in_=d_wv.ap().rearrange("(cc p) e -> p cc e", p=P))
        for sc in range(NSC):
            nc.sync.dma_start(out=xv_sb[:, :, sc * SCW:(sc + 1) * SCW],
                              in_=xv_ap[:, :, sc * SCW:(sc + 1) * SCW])
        for jt in range(NST):
            nc.sync.dma_start(out=maskT[:, jt, :], in_=msk_ap[:, jt, 0:SHW])

        # ---- projection unit emitters ----
        def proj_pair_steps(which, et, sc0):
            # transposed layout [e, m]: stationary = weight cols, moving = x.
            # Two s-chunks share each stationary load (ldweights amortize);
            # emitted as 4 per-cc steps (~0.5us each) to keep PE load even.
            if which == "q":
                x_sb, w_sb, bias = xq_sb, wq_sb, bq_sb
            else:
                x_sb, w_sb, bias = xk_sb, wk_sb, bk_sb
            state = {}

            def step(cc):
                def go():
                    if cc == 0:
                        state["A"] = projps.tile([P, SCW], f32, tag="pp",
                                                 name="ppA")
                        state["B"] = projps.tile([P, SCW], f32, tag="pp",
                                                 name="ppB")
                    psA, psB = state["A"], state["B"]
                    w_sl = w_sb[:, cc, et * P:(et + 1) * P]
                    nc.tensor.matmul(
                        psA, w_sl, x_sb[:, cc, sc0 * SCW:(sc0 + 1) * SCW],
                        start=(cc == 0), stop=(cc == NET - 1))
                    nc.tensor.matmul(
                        psB, w_sl, x_sb[:, cc, (sc0 + 1) * SCW:(sc0 + 2) * SCW],
                        start=(cc == 0), stop=(cc == NET - 1))
                return go

            def conv(ab):
                # conversion as its own step to spread engine load; k goes
                # to ScalarE (shares the exp table via identity, and the
                # Scalar engine has slack), q stays on DVE
                def go():
                    ps = state[ab]
                    sc = sc0 if ab == "A" else sc0 + 1
                    lo, hi = sc * SCW, (sc + 1) * SCW
                    if which == "q":
                        nc.vector.tensor_scalar(
                            qT[:, et, lo:hi], ps,
                            bias[:, et:et + 1], None, Add)
                    else:
                        he, ho = 2 * et, 2 * et + 1
                        nc.vector.tensor_scalar(
                            kT[0:64, he * S + lo:he * S + hi],
                            ps[0:64, :], bias[0:64, et:et + 1], None, Add)
                        nc.vector.tensor_scalar(
                            kT[64:P, ho * S + lo:ho * S + hi],
                            ps[64:P, :], bias[64:P, et:et + 1], None, Add)
                return go
            return [step(c) for c in range(NET)] + [conv("A"), conv("B")]

        def emit_now(steps):
            for s in steps:
                s()

        def v_unit(st):
            # natural layout [j, e] into v_aug rows for j-tile st
            ps = projps.tile([P, SCW], f32, tag="pp", name="pp")
            for cc in range(NET):
                nc.tensor.matmul(
                    ps,
                    xv_sb[:, cc, st * P:(st + 1) * P],
                    wv_sb[:, cc, :],
                    start=(cc == 0), stop=(cc == NET - 1))
            nc.vector.tensor_add(
                v_aug[:, st, :, 0:DK],
                ps.rearrange("p (h d) -> p h d", h=H),
                bv_bc.rearrange("p (h d) -> p h d", h=H))

        def final_unit(st):
            # out rows [st*128, (st+1)*128) @ Wo.T + bo -> DMA (fp16, 2 rings)
            ps = projps.tile([P, D], f32, tag="pp", name="pp")
            for cc in range(NET):
                nc.tensor.matmul(
                    ps,
                    outT[:, cc, st * P:(st + 1) * P],
                    wo_sb[:, cc, :],
                    start=(cc == 0), stop=(cc == NET - 1))
            o_sb = attn.tile([P, D], fp16, tag="os", bufs=2, name="o_sb")
            nc.vector.tensor_add(o_sb, ps, bo_bc)
            half = D // 2
            nc.sync.dma_start(
                out=d_out.ap()[st * P:(st + 1) * P, 0:half],
                in_=o_sb[:, 0:half])
            nc.sync.dma_start(
                out=d_out.ap()[st * P:(st + 1) * P, half:D],
                in_=o_sb[:, half:D])

        # ---- head phase: k et0, q et0 (first half) only; v-projection is
        # woven into the first head's slots (PV tolerates lag via pb bufs).
        # Emission order matches DMA arrival order.
        emit_now(proj_pair_steps("k", 0, 0))
        emit_now(proj_pair_steps("q", 0, 0))
        emit_now(proj_pair_steps("k", 0, 2))
        # wo / bo loads (needed from sh-boundary onwards)
        nc.sync.dma_start(
            out=wo_sb, in_=d_wo.ap().rearrange("(cc p) e -> p cc e", p=P))
        nc.sync.dma_start(
            out=bo_bc,
            in_=bass.AP(tensor=d_bo.ap().tensor, offset=0, ap=[[0, P], [1, D]]))
        # remaining xq loads (second half, needed by sh=1)
        for sc in range(2, NSC):
            nc.sync.dma_start(out=xq_sb[:, :, sc * SCW:(sc + 1) * SCW],
                              in_=xq_ap[:, :, sc * SCW:(sc + 1) * SCW])

        # ---- filler queue: remaining proj work, interleaved into slots ----
        # entries are ~0.5us per-cc steps, one per slot
        filler = []
        for et in range(1, NET):
            for sc0 in (0, 2):
                filler += proj_pair_steps("k", et, sc0)
            filler += proj_pair_steps("q", et, 0)
        for et in range(NET):
            filler += proj_pair_steps("q", et, 2)
        filler = [(fn, 1) for fn in filler]  # (emitter, slot spacing)
        fill_i = [0]
        cool = [0]

        def emit_filler():
            if cool[0] > 0:
                cool[0] -= 1
                return
            if fill_i[0] < len(filler):
                fn, spacing = filler[fill_i[0]]
                fill_i[0] += 1
                cool[0] = spacing - 1
                if fn is not None:
                    fn()

        # ---- attention ----
        for sh in range(NSH):
            c0 = sh * SHW
            for h in range(H):
                et, ro = h // 2, 64 * (h % 2)
                pv0 = attnps.tile([65, SCW], f32, tag="pv", bufs=2, name="pv0")
                pv1 = attnps.tile([65, SCW], f32, tag="pv", bufs=2, name="pv1")
                pvs = (pv0, pv1)
                # PV matmuls are deferred ~1 slot (2 for Pool-mul slots) so
                # the in-order PE stream never blocks on a pending mask-mul
                pvq = []
                for jt in range(NST):
                    sc_ps = attnps.tile([P, SHW], f32, tag="sc", bufs=2,
                                        name="sc_ps")
                    # stationary: head plane [128, 2, 128] whose second
                    # k-subtile is the shared zero plane; moving: paired qT
                    # with a stride-0 k-subtile dim (zero-weight rows kill
                    # the other head's values)
                    k_sl = kT[:, h * S + jt * P:h * S + (jt + 1) * P]
                    lhsT = bass.AP(
                        tensor=k_sl.tensor, offset=k_sl.offset,
                        ap=[k_sl.ap[0], [(H - h) * S, 2], k_sl.ap[-1]])
                    for i in range(2):
                        q_sl = qT[:, et, c0 + i * SCW:c0 + (i + 1) * SCW]
                        rhs = bass.AP(
                            tensor=q_sl.tensor, offset=q_sl.offset,
                            ap=[q_sl.ap[0], [0, 2], q_sl.ap[-1]])
                        nc.tensor.matmul(
                            sc_ps[:, i * SCW:(i + 1) * SCW],
                            lhsT, rhs,
                            start=True, stop=True, perf_mode=DR)
                    while pvq and pvq[0][0] <= jt:
                        pvq.pop(0)[1]()
                    ex = attn.tile([P, SHW], fp16, tag="ex", bufs=4, name="ex")
                    # 1/sqrt(dk)=1/8 applied here (q/k kept unscaled so the
                    # fp8 quantization stays in the normal-number range)
                    nc.scalar.activation(ex, sc_ps, Exp, scale=0.125)
                    pb = attn.tile([P, SHW], fp16, tag="pb", bufs=5, name="pb")
                    pool_mul = False
                    nc.vector.tensor_mul(pb, ex, maskT[:, jt, :])
                    if sh == 0 and h == 0:
                        v_unit(jt)
                    else:
                        emit_filler()

                    def mk_pv(jt=jt, pb=pb):
                        def go():
                            for i in range(2):
                                nc.tensor.matmul(
                                    pvs[i], v_aug[:, jt, h, :],
                                    pb[:, i * SCW:(i + 1) * SCW],
                                    start=(jt == 0), stop=(jt == NST - 1))
                        return go
                    pvq.append((jt + (2 if pool_mul else 1), mk_pv()))
                    if sh == 0 and h == 7:
                        # reload mask second half in place (WAR-tracked):
                        # mul(sh0,h7,jt) was the last reader of this slice.
                        nc.sync.dma_start(out=maskT[:, jt, :],
                                          in_=msk_ap[:, jt, SHW:S])
                for _, fn in pvq:
                    fn()
                # drain pv -> outT rows + denominator row (DVE; neither
                # GPSIMD nor DMA can read PSUM)
                for i in range(2):
                    cols = c0 + i * SCW
                    nc.vector.tensor_copy(
                        outT[ro:ro + DK, et, cols:cols + SCW], pvs[i][0:DK, :])
                    dst_t = attn.tile([1, SCW], f32, tag="dst", bufs=2,
                                      name="dst_t")
                    nc.vector.tensor_copy(dst_t, pvs[i][64:65, :])
                    pbase = h * 16 + i * 8
                    nc.gpsimd.dma_start(
                        out=denom[pbase:pbase + 8, sh, :], in_=dst_t)
                if h % 2 == 1:
                    # pair (2et, 2et+1) done for this half: normalize in place
                    rec = attn.tile([32, 64], fp16, tag="rec", bufs=2,
                                    name="rec")
                    with nc.allow_low_precision(reason="softmax recip fp16 ok"):
                        nc.vector.reciprocal(
                            rec, denom[et * 32:(et + 1) * 32, sh, :])
                    nc.sync.dma_start(
                        out=d_rec.ap()[2 * et:2 * et + 2, c0:c0 + SHW],
                        in_=rec)
                    rb = attn.tile([P, SHW], fp16, tag="rb", bufs=2,
                                   name="rb")
                    nc.gpsimd.dma_start(
                        out=rb[0:64, :],
                        in_=bass.AP(tensor=d_rec.ap().tensor,
                                    offset=(2 * et) * S + c0,
                                    ap=[[0, 64], [1, SHW]]))
                    nc.gpsimd.dma_start(
                        out=rb[64:128, :],
                        in_=bass.AP(tensor=d_rec.ap().tensor,
                                    offset=(2 * et + 1) * S + c0,
                                    ap=[[0, 64], [1, SHW]]))
                    nc.vector.tensor_mul(outT[:, et, c0:c0 + SHW],
                                         outT[:, et, c0:c0 + SHW], rb)
            # final projection for this s-half (sh0: as filler during sh1,
            # delayed a few slots so norm(sh0, et3) lands first)
            if sh == 0:
                filler.append((None, 6))
                for st in range(0, NST // NSH):
                    filler.append((lambda st=st: final_unit(st), 2))
        for st in range(NST // NSH, NST):
            final_unit(st)
        # drain any unemitted filler (shouldn't happen, but be safe)
        while fill_i[0] < len(filler):
            fn, _ = filler[fill_i[0]]
            fill_i[0] += 1
            if fn is not None:
                fn()

    nc.compile()
    return nc


def _get_nc():
    if "nc" not in _CACHE:
        _CACHE["nc"] = _build()
    return _CACHE["nc"]


def _preprocess(Q, K, V, mask, Wq, bq, Wk, bk, Wv, bv, Wo, bo):
    """Host-side sharding + layout marshalling (per-core input dicts)."""
    mT = np.ascontiguousarray(np.asarray(mask)[0, 0].T).astype(np.float16)
    wq_h = np.ascontiguousarray(np.asarray(Wq).T).astype(np.float16)
    wk_h = np.ascontiguousarray(np.asarray(Wk).T).astype(np.float16)
    wv_h = np.ascontiguousarray(np.asarray(Wv).T).astype(np.float16)
    wo_h = np.ascontiguousarray(np.asarray(Wo).T).astype(np.float16)
    bq_h = np.asarray(bq, dtype=np.float32)
    bk_h = np.asarray(bk, dtype=np.float32)
    bv_h = np.asarray(bv, dtype=np.float32)
    bo_h = np.asarray(bo, dtype=np.float32)
    Q, K, V = np.asarray(Q), np.asarray(K), np.asarray(V)
    in_maps = []
    for b in range(B):
        in_maps.append({
            "xq": np.ascontiguousarray(Q[b].T).astype(np.float16),
            "xk": np.ascontiguousarray(K[b].T).astype(np.float16),
            "xv": np.ascontiguousarray(V[b].T).astype(np.float16),
            "mskT": mT,
            "wq": wq_h, "wk": wk_h, "wv": wv_h, "wo": wo_h,
            "bq": bq_h, "bk": bk_h, "bv": bv_h, "bo": bo_h,
        })
    return in_maps


def run(inputs: dict, trace: bool = False):
    nc = _get_nc()
    in_maps = _preprocess(**inputs)
    res = run_bass_kernel_spmd(nc, in_maps, core_ids=list(range(B)), trace=trace)
    outp = np.stack([res.results[b]["out"] for b in range(B)], axis=0)
    return outp.astype(np.float32), res


def kernel(**inputs) -> np.ndarray:
    outp, _ = run(inputs, trace=False)
    return outp
